# revision 1
# baseline (speedup 1.0000x reference)
"""Trainium2 Bass kernel for a StyleGAN-style modulated conv2d.

Reference math (see problem statement):
    w  = kernel * he_std                       # equalized-lr
    s  = style @ w_mod + b_mod + 1             # [B, cin]
    s  = s / max|s|                            # global max-abs over [B, cin]
    w  = w * s[0][None, None, :, None]         # style[0] only -> one shared weight
    d  = rsqrt(sum(w^2, (0,1,2)) + 1e-8)
    w  = w * d
    y  = conv2d_same(x, w) + noise*(ns/2) + bias
    y  = lrelu(y, 0.2) * sqrt(2)

Because only style[0] modulates, the effective 3x3x128x128 weight is identical
for every batch element, so the device work is a plain 3x3 conv. The tiny
modulation math (a 512x128 matvec + norms, ~1e-6 of total FLOPs) is folded on
the host while sharding; the conv + activation run on 8 NeuronCores,
data-parallel over batch (1 image per core).

Device strategy per core:
  - x is pre-padded/transposed on the host to [cin=128, 258, 258] bf16 (zero
    SAME-padding baked in), so every DMA is a plain linear per-partition copy.
  - 3x3 conv = 9 accumulating matmuls per PSUM group: lhsT = w[cin,cout] per
    tap, rhs = shifted x rows ([2 rows x 256 cols] = 512 spatial AP), PSUM
    [cout=128, 512] fp32.
  - Epilogue on ScalarE: y = Lrelu(psum*sqrt2 + bias*sqrt2, alpha=0.2), which
    equals (lrelu(psum + bias))*sqrt2. The demod factor d is folded into the
    weights on the host (exactly as in the reference).
  - Output stays [cout, H*W] fp32 per core; host transposes back to NHWC.
"""

import math
from contextlib import ExitStack

import ml_dtypes
import numpy as np

import concourse.bacc as bacc
import concourse.bass as bass
import concourse.mybir as mybir
import concourse.tile as tile
from concourse.bass_utils import run_bass_kernel_spmd

B, H, W, CIN, COUT, KK, SDIM = 8, 256, 256, 128, 128, 3, 512
HP, WP = H + 2, W + 2  # zero-padded spatial dims (SAME padding for 3x3)
N_CORES = 8
ROWS_PER_SLAB = 32          # output rows per input slab
SLABS = H // ROWS_PER_SLAB  # 8
GROUP_ROWS = 2              # output rows per PSUM group (2*256 = 512 = 1 bank)
OUT_TILE_ROWS = 8           # rows per SBUF output tile (8*256*4B = 8KB/part)

BF16 = mybir.dt.bfloat16
F32 = mybir.dt.float32
SQRT2 = float(np.sqrt(np.float32(2.0)))


def _effective_weight(style, kernel, w_mod, b_mod):
    """Exactly the reference weight math, in fp32 numpy."""
    style = np.asarray(style, np.float32)
    kernel = np.asarray(kernel, np.float32)
    w_mod = np.asarray(w_mod, np.float32)
    b_mod = np.asarray(b_mod, np.float32)

    he_std = np.float32(1.0) / np.sqrt(np.float32(KK * KK * CIN))
    w = kernel * he_std
    s = (style @ w_mod + b_mod + np.float32(1.0)).astype(np.float32)
    s = s * (np.float32(1.0) / np.max(np.abs(s)))
    w = w * s[0][None, None, :, None]
    d = np.float32(1.0) / np.sqrt(
        np.sum(np.square(w), axis=(0, 1, 2), dtype=np.float32) + np.float32(1e-8)
    )
    w = w * d[None, None, None, :]
    return w.astype(np.float32)  # [3, 3, cin, cout]


def _build_program(with_noise: bool):
    # Bacc (not raw Bass): its compile() splits multi-sem sync waits into
    # event semaphores — TRN2 allows at most one wait per instruction.
    nc = bacc.Bacc(trn_type="TRN2")
    x = nc.declare_dram_parameter("x", [CIN, HP * WP], BF16, isOutput=False)
    w = nc.declare_dram_parameter("w", [CIN, 9 * COUT], BF16, isOutput=False)
    # ab[:,0] = bias*0.8*sqrt2, ab[:,1] = bias*0.2*sqrt2 (lrelu decomposition)
    ab = nc.declare_dram_parameter("ab", [COUT, 2], F32, isOutput=False)
    if with_noise:
        nz = nc.declare_dram_parameter("nz", [1, H * W], BF16, isOutput=False)
        ones = nc.declare_dram_parameter("ones", [1, COUT], BF16, isOutput=False)
    y = nc.declare_dram_parameter("y", [COUT, H * W], F32, isOutput=True)

    slab_rows_in = ROWS_PER_SLAB + 2  # input halo rows per slab

    with ExitStack() as ctx:
        tc = ctx.enter_context(tile.TileContext(nc))
        consts = ctx.enter_context(tc.tile_pool(name="consts", bufs=1))
        xpool = ctx.enter_context(tc.tile_pool(name="x", bufs=3))
        opool = ctx.enter_context(tc.tile_pool(name="out", bufs=3))
        pspool = ctx.enter_context(tc.tile_pool(name="ps", bufs=6, space="PSUM"))
        tpool = ctx.enter_context(tc.tile_pool(name="tmp", bufs=6))
        if with_noise:
            nzpool = ctx.enter_context(tc.tile_pool(name="nz", bufs=2))

        wt = consts.tile([CIN, 9 * COUT], BF16)
        nc.sync.dma_start(wt[:], w[:])
        abt = consts.tile([COUT, 2], F32)
        nc.sync.dma_start(abt[:], ab[:])
        if with_noise:
            onest = consts.tile([1, COUT], BF16)
            nc.sync.dma_start(onest[:], ones[:])

        for slab in range(SLABS):
            r0 = slab * ROWS_PER_SLAB  # first output row of the slab
            xt = xpool.tile([CIN, slab_rows_in * WP], BF16)
            nc.sync.dma_start(xt[:], x[:, r0 * WP : (r0 + slab_rows_in) * WP])
            xv = xt[:].rearrange("p (r c) -> p r c", c=WP)
            if with_noise:
                nzt = nzpool.tile([1, ROWS_PER_SLAB * W], BF16)
                nc.sync.dma_start(nzt[:], nz[:, r0 * W : (r0 + ROWS_PER_SLAB) * W])

            for half in range(ROWS_PER_SLAB // OUT_TILE_ROWS):
                ot = opool.tile([COUT, OUT_TILE_ROWS * W], F32)
                for g in range(OUT_TILE_ROWS // GROUP_ROWS):
                    rr = half * OUT_TILE_ROWS + g * GROUP_ROWS  # row in slab
                    ps = pspool.tile([COUT, GROUP_ROWS * W], F32)
                    for t in range(9):
                        dh, dw = divmod(t, 3)
                        rhs = xv[:, rr + dh : rr + dh + GROUP_ROWS, dw : dw + W]
                        nc.tensor.matmul(
                            ps[:],
                            wt[:, t * COUT : (t + 1) * COUT],
                            rhs,
                            start=(t == 0),
                            stop=(t == 8 and not with_noise),
                        )
                    if with_noise:
                        nc.tensor.matmul(
                            ps[:],
                            onest[:],
                            nzt[:, rr * W : (rr + GROUP_ROWS) * W],
                            start=False,
                            stop=True,
                        )
                    # sqrt2*lrelu(z,0.2) = Relu(0.8*sqrt2*z) + 0.2*sqrt2*z,
                    # z = psum + bias. ACT's Lrelu LUT has a fixed 0.01
                    # slope (alpha is ignored), so build it from exact ops.
                    oslice = ot[:, g * GROUP_ROWS * W : (g + 1) * GROUP_ROWS * W]
                    t1 = tpool.tile([COUT, GROUP_ROWS * W], F32)
                    nc.scalar.activation(
                        t1[:],
                        ps[:],
                        mybir.ActivationFunctionType.Relu,
                        bias=abt[:, 0:1],
                        scale=0.8 * SQRT2,
                    )
                    nc.scalar.activation(
                        oslice,
                        ps[:],
                        mybir.ActivationFunctionType.Identity,
                        bias=abt[:, 1:2],
                        scale=0.2 * SQRT2,
                    )
                    nc.vector.tensor_add(oslice, oslice, t1[:])
                row = r0 + half * OUT_TILE_ROWS
                nc.sync.dma_start(
                    y[:, row * W : (row + OUT_TILE_ROWS) * W], ot[:]
                )
    nc.finalize()  # Bacc.compile(): reg alloc + split multi-sem waits (TRN2)
    return nc


def _run(inputs, trace=False, **spmd_kwargs):
    x = np.asarray(inputs["x"])
    noise_strength = float(np.asarray(inputs["noise_strength"]).reshape(-1)[0])
    bias = np.asarray(inputs["bias"], np.float32)

    w_eff = _effective_weight(
        inputs["style"], inputs["kernel"], inputs["w_mod"], inputs["b_mod"]
    )
    # [3,3,cin,cout] -> [cin, tap*cout], tap-major free dim
    w_dev = np.ascontiguousarray(
        w_eff.transpose(2, 0, 1, 3).reshape(CIN, 9 * COUT)
    ).astype(ml_dtypes.bfloat16)

    # Pad + NHWC->NCHW per image, cast bf16. Zero borders bake in SAME padding.
    x_pad = np.zeros((B, CIN, HP, WP), dtype=ml_dtypes.bfloat16)
    x_pad[:, :, 1 : H + 1, 1 : W + 1] = x.transpose(0, 3, 1, 2).astype(
        ml_dtypes.bfloat16
    )

    ab = np.stack(
        [
            bias * np.float32(0.8 * SQRT2),
            bias * np.float32(0.2 * SQRT2),
        ],
        axis=1,
    ).astype(np.float32)  # [COUT, 2]

    with_noise = noise_strength != 0.0
    in_maps = []
    for b in range(B):
        m = {
            "x": np.ascontiguousarray(x_pad[b].reshape(CIN, HP * WP)),
            "w": w_dev,
            "ab": ab,
        }
        if with_noise:
            nzb = np.asarray(inputs["noise"], np.float32)[b, :, :, 0] * np.float32(
                noise_strength / 2.0
            )
            m["nz"] = nzb.reshape(1, H * W).astype(ml_dtypes.bfloat16)
            m["ones"] = np.ones((1, COUT), dtype=ml_dtypes.bfloat16)
        in_maps.append(m)

    nc = _build_program(with_noise)
    res = run_bass_kernel_spmd(
        nc, in_maps, list(range(N_CORES)), trace=trace, **spmd_kwargs
    )

    out = np.empty((B, H, W, COUT), dtype=np.float32)
    for b in range(B):
        out[b] = res.results[b]["y"].reshape(COUT, H, W).transpose(1, 2, 0)
    return out, res


def kernel(**inputs):
    out, _ = _run(inputs)
    return out

